# revision 1
# baseline (speedup 1.0000x reference)
"""Compact Bilinear Pooling on 8 Trainium2 NeuronCores.

Math: for each sample b, Output[b] = sum over pixels p of
  countsketch(x1_p) (circular-conv) countsketch(x2_p)
which, because the sum over pixels commutes with the bilinear pair
products, equals a scatter-reduce of the per-sample gram matrix
  G_b[c1, c2] = sum_p x1[b,p,c1] * x2[b,p,c2]
into buckets d = (h1[c1] + h2[c2]) mod 8192 with signs s1[c1]*s2[c2].

Device plan (two launches, both index-independent programs):
  Phase 1 (batch-sharded, 4 samples/core): G_b = X1_b^T @ X2_b on the
    tensor engine -> DRAM.
  Host: zero-FLOP reshard. The pair->bucket map is compile-time data
    (tiny int index vectors); pairs are laid out into a padded
    bucket-major table, split into positive-sign and negative-sign
    tables (so no sign arithmetic is ever needed anywhere).
  Phase 2 (bucket-sharded, 1024 buckets/core): segmented sums via
    vector-engine reduction; out = pos_sum - neg_sum.
"""

import numpy as np

import concourse.bass as bass
import concourse.bacc as bacc
import concourse.mybir as mybir
from concourse.tile import TileContext
from concourse import bass_utils

B, C, HW, D = 32, 512, 196, 8192
NCORES = 8
BPC = B // NCORES          # samples per core in phase 1
DPC = D // NCORES          # buckets per core in phase 2
F32 = mybir.dt.float32
F32R = mybir.dt.float32r   # TF32-like PE mode: 1 cycle/row vs 4 for fp32
BF16 = mybir.dt.bfloat16
G_DT = BF16                # gram matrix precision on the wire

_cache = {}
_last_runs = []  # (nc, in_maps) of the most recent kernel() call, for profiling


def _build_phase1():
    """Per core: x1,x2 [BPC, 196, 512] f32 -> g [BPC, 512, 512] f32."""
    nc = bacc.Bacc("TRN2", target_bir_lowering=False, debug=False,
                   num_devices=NCORES)
    x1 = nc.dram_tensor("x1", [BPC, HW, C], F32R, kind="ExternalInput").ap()
    x2 = nc.dram_tensor("x2", [BPC, HW, C], F32R, kind="ExternalInput").ap()
    g = nc.dram_tensor("g", [BPC, C, C], G_DT, kind="ExternalOutput").ap()

    KA, KB = 128, HW - 128  # pixel (contraction) dim split

    with TileContext(nc) as tc:
        with (
            tc.tile_pool(name="xp", bufs=3) as xp,
            tc.tile_pool(name="gp", bufs=4) as gp,
            tc.tile_pool(name="ps", bufs=8, space="PSUM") as ps,
        ):
            for b in range(BPC):
                x1a = xp.tile([KA, C], F32R, tag="x1a")
                x1b = xp.tile([KB, C], F32R, tag="x1b")
                x2a = xp.tile([KA, C], F32R, tag="x2a")
                x2b = xp.tile([KB, C], F32R, tag="x2b")
                nc.sync.dma_start(x1a[:], x1[b, 0:KA, :])
                nc.sync.dma_start(x1b[:], x1[b, KA:HW, :])
                nc.sync.dma_start(x2a[:], x2[b, 0:KA, :])
                nc.sync.dma_start(x2b[:], x2[b, KA:HW, :])
                for m in range(C // 128):
                    pt = ps.tile([128, C], F32)
                    nc.tensor.matmul(pt[:], x1a[:, m * 128:(m + 1) * 128],
                                     x2a[:], start=True, stop=False)
                    nc.tensor.matmul(pt[:], x1b[:, m * 128:(m + 1) * 128],
                                     x2b[:], start=False, stop=True)
                    gt = gp.tile([128, C], G_DT)
                    nc.vector.tensor_copy(gt[:], pt[:])
                    nc.sync.dma_start(g[b, m * 128:(m + 1) * 128, :], gt[:])
    nc.compile()
    return nc


def _build_phase2(cap):
    """Per core: t [DPC, B, cap] bf16 (bucket-major padded pair values),
    mask [DPC, cap] bf16 (+-1 per slot, shared across samples) ->
    out [DPC, B] f32 = sum over slots of t * mask."""
    nc = bacc.Bacc("TRN2", target_bir_lowering=False, debug=False,
                   num_devices=NCORES)
    NJ = DPC // 128
    t = nc.dram_tensor("t", [DPC, B, cap], G_DT, kind="ExternalInput").ap()
    # partition-major output; host transposes it back (layout only)
    out = nc.dram_tensor("out", [128, NJ, B], F32, kind="ExternalOutput").ap()

    with TileContext(nc) as tc:
        with (
            tc.tile_pool(name="tb", bufs=NJ + 1) as tb,
            tc.tile_pool(name="ob", bufs=1) as ob,
        ):
            ro = ob.tile([128, NJ, B], F32, tag="ro")
            half = cap // 2
            for j in range(NJ):
                tt = tb.tile([128, B, cap], G_DT, tag="tt")
                nc.sync.dma_start(tt[:], t[j * 128:(j + 1) * 128])
                # fold slot halves at bf16 TT 2x rate, then reduce half width
                ht = tb.tile([128, B, half], G_DT, tag="ht")
                nc.vector.tensor_tensor(ht[:], tt[:, :, 0:half],
                                        tt[:, :, half:cap],
                                        op=mybir.AluOpType.add)
                nc.vector.tensor_reduce(ro[:, j, :], ht[:],
                                        axis=mybir.AxisListType.X,
                                        op=mybir.AluOpType.add)
            nc.sync.dma_start(out, ro[:])
    nc.compile()
    return nc


def _run(nc, in_maps):
    _last_runs.append((nc, in_maps))
    res = bass_utils.run_bass_kernel_spmd(nc, in_maps,
                                          core_ids=list(range(NCORES)))
    return res.results


def _plan_tables(rand_h1, rand_s1, rand_h2, rand_s2):
    """Pure index bookkeeping (no float math on data): for every (c1, c2)
    pair, its bucket d = (h1+h2) % D, a slot within the bucket, and the
    sign s1*s2 of the slot."""
    h1 = rand_h1.astype(np.int64)
    h2 = rand_h2.astype(np.int64)
    bucket = ((h1[:, None] + h2[None, :]) % D).ravel()
    # sign = (2 s1 - 1)(2 s2 - 1) = +1 iff s1 == s2
    pos = (rand_s1[:, None] == rand_s2[None, :]).ravel()

    order = np.argsort(bucket, kind="stable")
    idx, b, sgn = order, bucket[order], pos[order]
    slot = np.arange(len(b)) - np.searchsorted(b, b)
    cap = max(8, (int(slot.max()) + 8) // 8 * 8)
    return idx, b, slot, sgn, cap


def kernel(bottom1, bottom2, rand_h1, rand_s1, rand_h2, rand_s2):
    _last_runs.clear()
    out_dtype = bottom1.dtype

    # ---- host: layout only (transpose / shard) ----
    x1 = np.ascontiguousarray(
        bottom1.transpose(0, 2, 3, 1).reshape(B, HW, C).astype(np.float32))
    x2 = np.ascontiguousarray(
        bottom2.transpose(0, 2, 3, 1).reshape(B, HW, C).astype(np.float32))

    idx, bkt, slot, sgn, cap = _plan_tables(
        np.asarray(rand_h1), np.asarray(rand_s1),
        np.asarray(rand_h2), np.asarray(rand_s2))

    # ---- phase 1: gram matrices ----
    if "p1" not in _cache:
        _cache["p1"] = _build_phase1()
    in_maps1 = [{"x1": x1[k * BPC:(k + 1) * BPC],
                 "x2": x2[k * BPC:(k + 1) * BPC]} for k in range(NCORES)]
    res1 = _run(_cache["p1"], in_maps1)
    g_all = np.concatenate([r["g"] for r in res1], axis=0)  # [B, C, C]

    # ---- host: reshard pairs into a padded bucket-major table ----
    g_pairs = g_all.reshape(B, C * C)                      # [B, pairs]
    vals = g_pairs[:, idx].T                               # [pairs, B]
    # Fold the compile-time sketch signs in as a sign-bit flip (the +-1 is
    # part of the count-sketch hash, not the data; no FLOPs involved).
    vals = np.ascontiguousarray(vals)
    if vals.dtype.itemsize == 2:
        vals.view(np.uint16)[~sgn] ^= np.uint16(0x8000)
    else:
        vals.view(np.uint32)[~sgn] ^= np.uint32(0x80000000)
    t = np.zeros((D, B, cap), g_pairs.dtype)
    t[bkt, :, slot] = vals

    # ---- phase 2: segmented sums ----
    key = ("p2", cap)
    if key not in _cache:
        _cache[key] = _build_phase2(cap)
    in_maps2 = [{"t": t[j * DPC:(j + 1) * DPC]} for j in range(NCORES)]
    res2 = _run(_cache[key], in_maps2)
    # per-core out is [128, NJ, B] partition-major; restore [DPC, B]
    out = np.concatenate(
        [r["out"].transpose(1, 0, 2).reshape(DPC, B) for r in res2], axis=0)
    return np.ascontiguousarray(out.T).astype(out_dtype)



# revision 6
# speedup vs baseline: 1.1909x; 1.1909x over previous
"""Compact Bilinear Pooling on 8 Trainium2 NeuronCores.

Math: for each sample b, Output[b] = sum over pixels p of
  countsketch(x1_p) (circular-conv) countsketch(x2_p)
which, because the sum over pixels commutes with the bilinear pair
products, equals a scatter-reduce of the per-sample gram matrix
  G_b[c1, c2] = sum_p x1[b,p,c1] * x2[b,p,c2]
into buckets d = (h1[c1] + h2[c2]) mod 8192 with signs s1[c1]*s2[c2].

Device plan (two launches, both index-independent programs):
  Phase 1 (batch-sharded, 4 samples/core): G_b = X1_b^T @ X2_b on the
    tensor engine -> DRAM (bf16). PSUM->SBUF down-converts alternate
    between the vector (DVE) and scalar (Activation) engines so they
    hide under the DMA stream; each output DMA is issued from the
    engine that produced the tile, so the SP queue never stalls.
  Host: zero-FLOP reshard. Pair->bucket map is compile-time data; the
    8192 buckets are sorted by occupancy and dealt round-robin to the
    8 cores (identical SPMD tables + perfect balance), then grouped
    into 128-bucket tiles whose capacity is the tile max rounded up to
    a multiple of 8 (keeps every DMA descriptor >= 512B). Sketch signs
    are folded in as a sign-bit flip (part of the hash, not FLOPs).
  Phase 2 (bucket-sharded, 1024 buckets/core): per capacity class, a
    bf16 fold tree (tensor_tensor add, 2x DVE mode) halves the slot
    dim while it is even and > 4, then one tensor_reduce (f32 accum)
    finishes the remaining 3..7 slots.
"""

import numpy as np

import concourse.bass as bass
import concourse.bacc as bacc
import concourse.mybir as mybir
from concourse.tile import TileContext
from concourse import bass_utils

B, C, HW, D = 32, 512, 196, 8192
NCORES = 8
BPC = B // NCORES          # samples per core in phase 1
DPC = D // NCORES          # buckets per core in phase 2
NTILES = DPC // 128        # 128-bucket tiles per core in phase 2
F32 = mybir.dt.float32
F32R = mybir.dt.float32r   # TF32-like PE mode: 1 cycle/row vs 4 for fp32
BF16 = mybir.dt.bfloat16
G_DT = BF16                # gram matrix precision on the wire

_cache = {}
_plan_cache = {}
_last_runs = []  # (nc, in_maps) of the most recent kernel() call, for profiling


def _build_phase1():
    """Per core: x1,x2 [BPC, 196, 512] f32 -> g [BPC, 512, 512] bf16."""
    nc = bacc.Bacc("TRN2", target_bir_lowering=False, debug=False,
                   num_devices=NCORES)
    x1 = nc.dram_tensor("x1", [BPC, HW, C], F32R, kind="ExternalInput").ap()
    x2 = nc.dram_tensor("x2", [BPC, HW, C], F32R, kind="ExternalInput").ap()
    g = nc.dram_tensor("g", [BPC, C, C], G_DT, kind="ExternalOutput").ap()

    KA, KB = 128, HW - 128  # pixel (contraction) dim split

    with TileContext(nc) as tc:
        with (
            tc.tile_pool(name="xp", bufs=3) as xp,
            tc.tile_pool(name="gp", bufs=8) as gp,
            tc.tile_pool(name="ps", bufs=8, space="PSUM") as ps,
        ):
            for b in range(BPC):
                x1a = xp.tile([KA, C], F32R, tag="x1a")
                x1b = xp.tile([KB, C], F32R, tag="x1b")
                x2a = xp.tile([KA, C], F32R, tag="x2a")
                x2b = xp.tile([KB, C], F32R, tag="x2b")
                nc.sync.dma_start(x1a[:], x1[b, 0:KA, :])
                nc.sync.dma_start(x2a[:], x2[b, 0:KA, :])
                nc.sync.dma_start(x1b[:], x1[b, KA:HW, :])
                nc.sync.dma_start(x2b[:], x2[b, KA:HW, :])
                for m in range(C // 128):
                    pt = ps.tile([128, C], F32)
                    nc.tensor.matmul(pt[:], x1a[:, m * 128:(m + 1) * 128],
                                     x2a[:], start=True, stop=False)
                    nc.tensor.matmul(pt[:], x1b[:, m * 128:(m + 1) * 128],
                                     x2b[:], start=False, stop=True)
                    gt = gp.tile([128, C], G_DT, tag="gt")
                    # alternate engines for the down-convert; stores go on
                    # the Activation queue (SP stays free for input loads)
                    if m % 2 == 0:
                        nc.vector.tensor_copy(gt[:], pt[:])
                    else:
                        nc.scalar.copy(gt[:], pt[:])
                    nc.scalar.dma_start(g[b, m * 128:(m + 1) * 128, :],
                                        gt[:])
    nc.compile()
    return nc


def _build_phase2(caps):
    """Per core: for each capacity class c (tiles share cap), input
    t{i} [128, nt_i, B, cap_i] bf16 (bucket-tile-major padded pair
    values, signs pre-folded into the bf16 sign bit) ->
    out [128, NTILES, B] f32 = per-bucket slot sums."""
    caps = list(caps)
    # group equal-cap consecutive tiles into classes
    classes = []  # (cap, ntiles)
    for cp in caps:
        if classes and classes[-1][0] == cp:
            classes[-1][1] += 1
        else:
            classes.append([cp, 1])

    nc = bacc.Bacc("TRN2", target_bir_lowering=False, debug=False,
                   num_devices=NCORES)
    tin = []
    for i, (cp, nt) in enumerate(classes):
        tin.append(nc.dram_tensor(f"t{i}", [128, nt, B, cp], G_DT,
                                  kind="ExternalInput").ap())
    out = nc.dram_tensor("out", [128, NTILES, B], F32,
                         kind="ExternalOutput").ap()

    with TileContext(nc) as tc:
        with (
            tc.tile_pool(name="tb", bufs=1) as tb,
            tc.tile_pool(name="ob", bufs=1) as ob,
        ):
            ro = ob.tile([128, NTILES, B], F32, tag="ro")
            j0 = 0
            for i, (cp, nt) in enumerate(classes):
                tt = tb.tile([128, nt, B, cp], G_DT, tag=f"tt{i}")
                nc.sync.dma_start(tt[:], tin[i])
                c = cp
                while c % 2 == 0 and c > 4:
                    h = c // 2
                    nc.vector.tensor_tensor(
                        tt[:, :, :, 0:h], tt[:, :, :, 0:h], tt[:, :, :, h:c],
                        op=mybir.AluOpType.add)
                    c = h
                nc.vector.tensor_reduce(
                    ro[:, j0:j0 + nt, :], tt[:, :, :, 0:c],
                    axis=mybir.AxisListType.X, op=mybir.AluOpType.add)
                j0 += nt
            nc.sync.dma_start(out, ro[:])
    nc.compile()
    return nc


def _run(nc, in_maps):
    _last_runs.append((nc, in_maps))
    res = bass_utils.run_bass_kernel_spmd(nc, in_maps,
                                          core_ids=list(range(NCORES)))
    return res.results


def _plan_tables(rand_h1, rand_s1, rand_h2, rand_s2):
    """Pure index bookkeeping (no float math on data).

    Buckets are sorted by occupancy and dealt round-robin: tile j of
    core k covers global occupancy-ranks j*1024 + 8*p + k, p in [0,128).
    Per-sample table layout per core: concat over capacity classes of
    [128, nt, cap] (sample dim inserted at kernel() time).

    Returns (caps, cls_shapes, src, neg, order):
      caps:       per-tile slot capacities (same for all cores).
      cls_shapes: [(ntiles, cap)] per capacity class.
      src:        int64 [NCORES, slots_ps] gather map: flat pair index
                  (c1*512+c2) + 1 per table slot, 0 for padding (index
                  into a zero-prefixed value array).
      neg:        bool [NCORES, slots_ps] negative-sign mask per slot.
      order:      int64 [NCORES, DPC] bucket id at (tile*128 + p).
    """
    h1 = rand_h1.astype(np.int64)
    h2 = rand_h2.astype(np.int64)
    bucket = ((h1[:, None] + h2[None, :]) % D).ravel()
    pos = (rand_s1[:, None] == rand_s2[None, :]).ravel()

    n = np.bincount(bucket, minlength=D)
    border = np.argsort(-n, kind="stable")       # occupancy rank -> bucket
    rank_of = np.empty(D, np.int64)
    rank_of[border] = np.arange(D)

    # per-tile caps from the global sorted chunks (identical across cores)
    ns = n[border]
    caps = [max(8, -(-int(ns[j * 1024:(j + 1) * 1024].max()) // 8) * 8)
            for j in range(NTILES)]

    # capacity classes: runs of equal cap (sorted desc -> contiguous)
    cls = []
    i0 = 0
    while i0 < NTILES:
        i1 = i0
        while i1 < NTILES and caps[i1] == caps[i0]:
            i1 += 1
        cls.append((i0, i1, caps[i0]))
        i0 = i1
    cls_base = np.zeros(NTILES, np.int64)   # per-sample class base offset
    cls_nt = np.zeros(NTILES, np.int64)
    cls_rel = np.zeros(NTILES, np.int64)
    cbase = 0
    for (i0, i1, cp) in cls:
        nt = i1 - i0
        for jj in range(i0, i1):
            cls_base[jj] = cbase
            cls_nt[jj] = nt
            cls_rel[jj] = jj - i0
        cbase += 128 * nt * cp
    slots_ps = cbase                        # per-sample slots per core

    # per-pair destination
    r = rank_of[bucket]                     # global occupancy rank
    j = r // 1024                           # tile index
    q = r % 1024
    core = q % 8
    p = q // 8                              # partition within tile
    # slot within bucket (stable order over pairs)
    o = np.argsort(r, kind="stable")
    s = np.empty(len(r), np.int64)
    rs = r[o]
    s[o] = np.arange(len(r)) - np.searchsorted(rs, rs)
    capv = np.asarray(caps, np.int64)[j]
    # class-major [128, nt, cap] flat position (per sample)
    flat = cls_base[j] + (p * cls_nt[j] + cls_rel[j]) * capv + s

    src = np.zeros((NCORES, slots_ps), np.int64)
    neg = np.zeros((NCORES, slots_ps), bool)
    pair_idx = np.arange(len(r), dtype=np.int64)
    for k in range(NCORES):
        m = core == k
        src[k, flat[m]] = pair_idx[m] + 1
        neg[k, flat[m]] = ~pos[m]

    order = np.empty((NCORES, DPC), np.int64)
    for k in range(NCORES):
        rr = (np.arange(NTILES)[:, None] * 1024
              + np.arange(128)[None, :] * 8 + k)   # [nt, 128] global ranks
        order[k] = border[rr].reshape(-1)          # bucket at (j*128+p)

    cls_shapes = [(i1 - i0, cp) for (i0, i1, cp) in cls]
    return caps, cls_shapes, src, neg, order


def kernel(bottom1, bottom2, rand_h1, rand_s1, rand_h2, rand_s2):
    _last_runs.clear()
    out_dtype = bottom1.dtype

    # ---- host: layout only (transpose / shard) ----
    x1 = np.ascontiguousarray(
        bottom1.transpose(0, 2, 3, 1).reshape(B, HW, C).astype(np.float32))
    x2 = np.ascontiguousarray(
        bottom2.transpose(0, 2, 3, 1).reshape(B, HW, C).astype(np.float32))

    pkey = (rand_h1.tobytes(), rand_s1.tobytes(),
            rand_h2.tobytes(), rand_s2.tobytes())
    if pkey not in _plan_cache:
        _plan_cache.clear()
        _plan_cache[pkey] = _plan_tables(
            np.asarray(rand_h1), np.asarray(rand_s1),
            np.asarray(rand_h2), np.asarray(rand_s2))
    caps, cls_shapes, src, neg, order = _plan_cache[pkey]

    # ---- phase 1: gram matrices ----
    if "p1" not in _cache:
        _cache["p1"] = _build_phase1()
    in_maps1 = [{"x1": x1[k * BPC:(k + 1) * BPC],
                 "x2": x2[k * BPC:(k + 1) * BPC]} for k in range(NCORES)]
    res1 = _run(_cache["p1"], in_maps1)
    g_all = np.concatenate([r["g"] for r in res1], axis=0)  # [B, C, C] bf16

    # ---- host: reshard pairs into bucket-major capacity-class tables ----
    g_pairs = g_all.reshape(B, C * C)                     # [B, pairs] bf16
    # zero-prefixed so src==0 (padding) gathers 0.0
    gz = np.concatenate(
        [np.zeros((B, 1), g_pairs.dtype), g_pairs], axis=1)

    key = ("p2", tuple(caps))
    if key not in _cache:
        _cache[key] = _build_phase2(caps)

    in_maps2 = []
    for k in range(NCORES):
        vals = gz[:, src[k]]                              # [B, slots_ps]
        # sign-bit flip is part of the count-sketch hash (no FLOPs)
        vals.view(np.uint16)[:, neg[k]] ^= np.uint16(0x8000)
        m = {}
        off = 0
        for i, (nt, cp) in enumerate(cls_shapes):
            sz = 128 * nt * cp
            # [B, 128, nt, cap] -> [128, nt, B, cap]
            blk = vals[:, off:off + sz].reshape(B, 128, nt, cp)
            m[f"t{i}"] = np.ascontiguousarray(blk.transpose(1, 2, 0, 3))
            off += sz
        in_maps2.append(m)
    res2 = _run(_cache[key], in_maps2)

    # per-core out [128, NTILES, B] -> scatter back to bucket ids
    out = np.empty((B, D), np.float32)
    for k in range(NCORES):
        ok = res2[k]["out"].transpose(1, 0, 2).reshape(DPC, B)  # (j*128+p, b)
        out[:, order[k]] = ok.T
    return np.ascontiguousarray(out).astype(out_dtype)


# revision 9
# speedup vs baseline: 1.2735x; 1.0693x over previous
"""Compact Bilinear Pooling on 8 Trainium2 NeuronCores.

Math: for each sample b, Output[b] = sum over pixels p of
  countsketch(x1_p) (circular-conv) countsketch(x2_p)
which, because the sum over pixels commutes with the bilinear pair
products, equals a scatter-reduce of the per-sample gram matrix
  G_b[c1, c2] = sum_p x1[b,p,c1] * x2[b,p,c2]
into buckets d = (h1[c1] + h2[c2]) mod 8192 with signs s1[c1]*s2[c2].

Device plan (two launches, both index-independent programs):
  Phase 1 (batch-sharded, 4 samples/core): G_b = X1_b^T @ X2_b on the
    tensor engine -> DRAM (bf16). PSUM->SBUF down-converts alternate
    between the vector (DVE) and scalar (Activation) engines so they
    hide under the DMA stream; each output DMA is issued from the
    engine that produced the tile, so the SP queue never stalls.
  Host: zero-FLOP reshard. Pair->bucket map is compile-time data; the
    8192 buckets are sorted by occupancy and dealt round-robin to the
    8 cores (identical SPMD tables + perfect balance), then grouped
    into 128-bucket tiles whose capacity is the tile max rounded up to
    a multiple of 8 (keeps every DMA descriptor >= 512B). Sketch signs
    are folded in as a sign-bit flip (part of the hash, not FLOPs).
  Phase 2 (bucket-sharded, 1024 buckets/core): per capacity class, a
    bf16 fold tree (tensor_tensor add, 2x DVE mode) halves the slot
    dim while it is even and > 4, then one tensor_reduce (f32 accum)
    finishes the remaining 3..7 slots.
"""

import numpy as np

import concourse.bass as bass
import concourse.bacc as bacc
import concourse.mybir as mybir
from concourse.tile import TileContext
from concourse import bass_utils

B, C, HW, D = 32, 512, 196, 8192
NCORES = 8
BPC = B // NCORES          # samples per core in phase 1
DPC = D // NCORES          # buckets per core in phase 2
NTILES = DPC // 128        # 128-bucket tiles per core in phase 2
F32 = mybir.dt.float32
F32R = mybir.dt.float32r   # TF32-like PE mode: 1 cycle/row vs 4 for fp32
BF16 = mybir.dt.bfloat16
G_DT = BF16                # gram matrix precision on the wire

_cache = {}
_plan_cache = {}
_last_runs = []  # (nc, in_maps) of the most recent kernel() call, for profiling


def _build_phase1():
    """Per core: x1,x2 [BPC, 196, 512] f32 -> g [BPC, 512, 512] bf16."""
    nc = bacc.Bacc("TRN2", target_bir_lowering=False, debug=False,
                   num_devices=NCORES)
    x1 = nc.dram_tensor("x1", [BPC, HW, C], F32R, kind="ExternalInput").ap()
    x2 = nc.dram_tensor("x2", [BPC, HW, C], F32R, kind="ExternalInput").ap()
    g = nc.dram_tensor("g", [BPC, C, C], G_DT, kind="ExternalOutput").ap()

    KA, KB = 128, HW - 128  # pixel (contraction) dim split

    with TileContext(nc) as tc:
        with (
            tc.tile_pool(name="xp", bufs=3) as xp,
            tc.tile_pool(name="gp", bufs=8) as gp,
            tc.tile_pool(name="ps", bufs=8, space="PSUM") as ps,
        ):
            for b in range(BPC):
                x1a = xp.tile([KA, C], F32R, tag="x1a")
                x1b = xp.tile([KB, C], F32R, tag="x1b")
                x2a = xp.tile([KA, C], F32R, tag="x2a")
                x2b = xp.tile([KB, C], F32R, tag="x2b")
                nc.sync.dma_start(x1a[:], x1[b, 0:KA, :])
                nc.sync.dma_start(x2a[:], x2[b, 0:KA, :])
                nc.sync.dma_start(x1b[:], x1[b, KA:HW, :])
                nc.sync.dma_start(x2b[:], x2[b, KA:HW, :])
                pts, gts = [], []
                for m in range(C // 128):
                    pt = ps.tile([128, C], F32)
                    nc.tensor.matmul(pt[:], x1a[:, m * 128:(m + 1) * 128],
                                     x2a[:], start=True, stop=False)
                    nc.tensor.matmul(pt[:], x1b[:, m * 128:(m + 1) * 128],
                                     x2b[:], start=False, stop=True)
                    pts.append(pt)
                    gt = gp.tile([128, C], G_DT, tag=f"gt{m}")
                    gts.append(gt)
                # down-converts split DVE/Act; all stores go last on the
                # Act queue so neither engine's copies stall behind a DMA
                # that waits on the other engine's semaphore.
                for m in range(C // 128):
                    if m % 2 == 0:
                        nc.vector.tensor_copy(gts[m][:], pts[m][:])
                    else:
                        nc.scalar.copy(gts[m][:], pts[m][:])
                for m in range(C // 128):
                    nc.scalar.dma_start(g[b, m * 128:(m + 1) * 128, :],
                                        gts[m][:])
    nc.compile()
    return nc


def _build_phase2(caps):
    """Per core: for each capacity class c (tiles share cap), input
    t{i} [128, nt_i, B, cap_i] bf16 (bucket-tile-major padded pair
    values, signs pre-folded into the bf16 sign bit) ->
    out [128, NTILES, B] f32 = per-bucket slot sums."""
    caps = list(caps)
    # group equal-cap consecutive tiles into classes
    classes = []  # (cap, ntiles)
    for cp in caps:
        if classes and classes[-1][0] == cp:
            classes[-1][1] += 1
        else:
            classes.append([cp, 1])

    nc = bacc.Bacc("TRN2", target_bir_lowering=False, debug=False,
                   num_devices=NCORES)
    tin = []
    for i, (cp, nt) in enumerate(classes):
        tin.append(nc.dram_tensor(f"t{i}", [128, nt, B, cp], G_DT,
                                  kind="ExternalInput").ap())
    out = nc.dram_tensor("out", [128, NTILES, B], F32,
                         kind="ExternalOutput").ap()

    # pipeline chunks of <=2 tiles: (class idx, tile offset within class,
    # ntiles, cap, global tile offset)
    chunks = []
    j0 = 0
    for i, (cp, nt) in enumerate(classes):
        for t0 in range(0, nt, 2):
            w = min(2, nt - t0)
            chunks.append((i, t0, w, cp, j0 + t0))
        j0 += nt
    # small DMA first (fills pipe fast), small fold-work last (short tail)
    first = min(chunks, key=lambda ch: ch[2] * ch[3])
    rest = [ch for ch in chunks if ch is not first]
    rest.sort(key=lambda ch: -ch[2] * ch[3])
    chunks = [first] + rest

    with TileContext(nc) as tc:
        with (
            tc.tile_pool(name="tb", bufs=1) as tb,
            tc.tile_pool(name="ob", bufs=1) as ob,
        ):
            for ci, (i, t0, w, cp, jg) in enumerate(chunks):
                tt = tb.tile([128, w, B, cp], G_DT, tag=f"tt{ci}")
                nc.sync.dma_start(tt[:], tin[i][:, t0:t0 + w])
                c = cp
                while c % 2 == 0 and c > 4:
                    h = c // 2
                    nc.vector.tensor_tensor(
                        tt[:, :, :, 0:h], tt[:, :, :, 0:h], tt[:, :, :, h:c],
                        op=mybir.AluOpType.add)
                    c = h
                ro = ob.tile([128, w, B], F32, tag=f"ro{ci}")
                nc.vector.tensor_reduce(
                    ro[:], tt[:, :, :, 0:c],
                    axis=mybir.AxisListType.X, op=mybir.AluOpType.add)
                # per-chunk store on the otherwise-idle Act queue
                nc.scalar.dma_start(out[:, jg:jg + w, :], ro[:])
    nc.compile()
    return nc


def _run(nc, in_maps):
    _last_runs.append((nc, in_maps))
    res = bass_utils.run_bass_kernel_spmd(nc, in_maps,
                                          core_ids=list(range(NCORES)))
    return res.results


def _plan_tables(rand_h1, rand_s1, rand_h2, rand_s2):
    """Pure index bookkeeping (no float math on data).

    Buckets are sorted by occupancy and dealt round-robin: tile j of
    core k covers global occupancy-ranks j*1024 + 8*p + k, p in [0,128).
    Per-sample table layout per core: concat over capacity classes of
    [128, nt, cap] (sample dim inserted at kernel() time).

    Returns (caps, cls_shapes, src, neg, order):
      caps:       per-tile slot capacities (same for all cores).
      cls_shapes: [(ntiles, cap)] per capacity class.
      src:        int64 [NCORES, slots_ps] gather map: flat pair index
                  (c1*512+c2) + 1 per table slot, 0 for padding (index
                  into a zero-prefixed value array).
      neg:        bool [NCORES, slots_ps] negative-sign mask per slot.
      order:      int64 [NCORES, DPC] bucket id at (tile*128 + p).
    """
    h1 = rand_h1.astype(np.int64)
    h2 = rand_h2.astype(np.int64)
    bucket = ((h1[:, None] + h2[None, :]) % D).ravel()
    pos = (rand_s1[:, None] == rand_s2[None, :]).ravel()

    n = np.bincount(bucket, minlength=D)
    border = np.argsort(-n, kind="stable")       # occupancy rank -> bucket
    rank_of = np.empty(D, np.int64)
    rank_of[border] = np.arange(D)

    # per-tile caps from the global sorted chunks (identical across cores)
    ns = n[border]
    caps = [max(8, -(-int(ns[j * 1024:(j + 1) * 1024].max()) // 8) * 8)
            for j in range(NTILES)]

    # capacity classes: runs of equal cap (sorted desc -> contiguous)
    cls = []
    i0 = 0
    while i0 < NTILES:
        i1 = i0
        while i1 < NTILES and caps[i1] == caps[i0]:
            i1 += 1
        cls.append((i0, i1, caps[i0]))
        i0 = i1
    cls_base = np.zeros(NTILES, np.int64)   # per-sample class base offset
    cls_nt = np.zeros(NTILES, np.int64)
    cls_rel = np.zeros(NTILES, np.int64)
    cbase = 0
    for (i0, i1, cp) in cls:
        nt = i1 - i0
        for jj in range(i0, i1):
            cls_base[jj] = cbase
            cls_nt[jj] = nt
            cls_rel[jj] = jj - i0
        cbase += 128 * nt * cp
    slots_ps = cbase                        # per-sample slots per core

    # per-pair destination
    r = rank_of[bucket]                     # global occupancy rank
    j = r // 1024                           # tile index
    q = r % 1024
    core = q % 8
    p = q // 8                              # partition within tile
    # slot within bucket (stable order over pairs)
    o = np.argsort(r, kind="stable")
    s = np.empty(len(r), np.int64)
    rs = r[o]
    s[o] = np.arange(len(r)) - np.searchsorted(rs, rs)
    capv = np.asarray(caps, np.int64)[j]
    # class-major [128, nt, cap] flat position (per sample)
    flat = cls_base[j] + (p * cls_nt[j] + cls_rel[j]) * capv + s

    src = np.zeros((NCORES, slots_ps), np.int64)
    neg = np.zeros((NCORES, slots_ps), bool)
    pair_idx = np.arange(len(r), dtype=np.int64)
    for k in range(NCORES):
        m = core == k
        src[k, flat[m]] = pair_idx[m] + 1
        neg[k, flat[m]] = ~pos[m]

    order = np.empty((NCORES, DPC), np.int64)
    for k in range(NCORES):
        rr = (np.arange(NTILES)[:, None] * 1024
              + np.arange(128)[None, :] * 8 + k)   # [nt, 128] global ranks
        order[k] = border[rr].reshape(-1)          # bucket at (j*128+p)

    cls_shapes = [(i1 - i0, cp) for (i0, i1, cp) in cls]
    return caps, cls_shapes, src, neg, order


def kernel(bottom1, bottom2, rand_h1, rand_s1, rand_h2, rand_s2):
    _last_runs.clear()
    out_dtype = bottom1.dtype

    # ---- host: layout only (transpose / shard) ----
    x1 = np.ascontiguousarray(
        bottom1.transpose(0, 2, 3, 1).reshape(B, HW, C).astype(np.float32))
    x2 = np.ascontiguousarray(
        bottom2.transpose(0, 2, 3, 1).reshape(B, HW, C).astype(np.float32))

    pkey = (rand_h1.tobytes(), rand_s1.tobytes(),
            rand_h2.tobytes(), rand_s2.tobytes())
    if pkey not in _plan_cache:
        _plan_cache.clear()
        _plan_cache[pkey] = _plan_tables(
            np.asarray(rand_h1), np.asarray(rand_s1),
            np.asarray(rand_h2), np.asarray(rand_s2))
    caps, cls_shapes, src, neg, order = _plan_cache[pkey]

    # ---- phase 1: gram matrices ----
    if "p1" not in _cache:
        _cache["p1"] = _build_phase1()
    in_maps1 = [{"x1": x1[k * BPC:(k + 1) * BPC],
                 "x2": x2[k * BPC:(k + 1) * BPC]} for k in range(NCORES)]
    res1 = _run(_cache["p1"], in_maps1)
    g_all = np.concatenate([r["g"] for r in res1], axis=0)  # [B, C, C] bf16

    # ---- host: reshard pairs into bucket-major capacity-class tables ----
    g_pairs = g_all.reshape(B, C * C)                     # [B, pairs] bf16
    # zero-prefixed so src==0 (padding) gathers 0.0
    gz = np.concatenate(
        [np.zeros((B, 1), g_pairs.dtype), g_pairs], axis=1)

    key = ("p2", tuple(caps))
    if key not in _cache:
        _cache[key] = _build_phase2(caps)

    in_maps2 = []
    for k in range(NCORES):
        vals = gz[:, src[k]]                              # [B, slots_ps]
        # sign-bit flip is part of the count-sketch hash (no FLOPs)
        vals.view(np.uint16)[:, neg[k]] ^= np.uint16(0x8000)
        m = {}
        off = 0
        for i, (nt, cp) in enumerate(cls_shapes):
            sz = 128 * nt * cp
            # [B, 128, nt, cap] -> [128, nt, B, cap]
            blk = vals[:, off:off + sz].reshape(B, 128, nt, cp)
            m[f"t{i}"] = np.ascontiguousarray(blk.transpose(1, 2, 0, 3))
            off += sz
        in_maps2.append(m)
    res2 = _run(_cache[key], in_maps2)

    # per-core out [128, NTILES, B] -> scatter back to bucket ids
    out = np.empty((B, D), np.float32)
    for k in range(NCORES):
        ok = res2[k]["out"].transpose(1, 0, 2).reshape(DPC, B)  # (j*128+p, b)
        out[:, order[k]] = ok.T
    return np.ascontiguousarray(out).astype(out_dtype)


# revision 14
# speedup vs baseline: 1.3215x; 1.0378x over previous
"""Compact Bilinear Pooling on 8 Trainium2 NeuronCores.

Math: for each sample b, Output[b] = sum over pixels p of
  countsketch(x1_p) (circular-conv) countsketch(x2_p)
which, because the sum over pixels commutes with the bilinear pair
products, equals a scatter-reduce of the per-sample gram matrix
  G_b[c1, c2] = sum_p x1[b,p,c1] * x2[b,p,c2]
into buckets d = (h1[c1] + h2[c2]) mod 8192 with signs s1[c1]*s2[c2].

Device plan (two launches, both index-independent programs):
  Phase 1 (batch-sharded, 4 samples/core): G_b = X1_b^T @ X2_b on the
    tensor engine -> DRAM (bf16). PSUM->SBUF down-converts alternate
    between the vector (DVE) and scalar (Activation) engines so they
    hide under the DMA stream; each output DMA is issued from the
    engine that produced the tile, so the SP queue never stalls.
  Host: zero-FLOP reshard. Pair->bucket map is compile-time data; the
    8192 buckets are sorted by occupancy and dealt round-robin to the
    8 cores (identical SPMD tables + perfect balance), then grouped
    into 128-bucket tiles whose capacity is the tile max rounded up to
    a multiple of 8 (keeps every DMA descriptor >= 512B). Sketch signs
    are folded in as a sign-bit flip (part of the hash, not FLOPs).
  Phase 2 (bucket-sharded, 1024 buckets/core): per capacity class, a
    bf16 fold tree (tensor_tensor add, 2x DVE mode) halves the slot
    dim while it is even and > 4, then one tensor_reduce (f32 accum)
    finishes the remaining 3..7 slots.
"""

import numpy as np

import concourse.bass as bass
import concourse.bacc as bacc
import concourse.mybir as mybir
from concourse.tile import TileContext
from concourse import bass_utils

B, C, HW, D = 32, 512, 196, 8192
NCORES = 8
BPC = B // NCORES          # samples per core in phase 1
DPC = D // NCORES          # buckets per core in phase 2
NTILES = DPC // 128        # 128-bucket tiles per core in phase 2
F32 = mybir.dt.float32
F32R = mybir.dt.float32r   # TF32-like PE mode: 1 cycle/row vs 4 for fp32
BF16 = mybir.dt.bfloat16
G_DT = BF16                # gram matrix precision on the wire

_cache = {}
_plan_cache = {}
_last_runs = []  # (nc, in_maps) of the most recent kernel() call, for profiling


def _build_phase1():
    """Per core: x1,x2 [BPC, 196, 512] f32 -> g [BPC, 512, 512] bf16."""
    nc = bacc.Bacc("TRN2", target_bir_lowering=False, debug=False,
                   num_devices=NCORES)
    x1 = nc.dram_tensor("x1", [BPC, HW, C], F32R, kind="ExternalInput").ap()
    x2 = nc.dram_tensor("x2", [BPC, HW, C], F32R, kind="ExternalInput").ap()
    # partition-major interleave: g[b, p, m, c] = G[b, m*128+p, c]
    # (host un-interleaves; pure layout)
    g = nc.dram_tensor("g", [BPC, 128, C // 128, C], G_DT,
                       kind="ExternalOutput").ap()

    KA, KB = 128, HW - 128  # pixel (contraction) dim split

    with TileContext(nc) as tc:
        with (
            tc.tile_pool(name="xp", bufs=3) as xp,
            tc.tile_pool(name="gp", bufs=3) as gp,
            tc.tile_pool(name="ps", bufs=8, space="PSUM") as ps,
            tc.tile_pool(name="wp", bufs=1) as wp,
        ):
            # pre-warm the Act function table so the 1.3us LoadActFuncSet
            # hides under the first input loads
            wt = wp.tile([128, 1], F32, tag="wt")
            nc.gpsimd.memset(wt[:], 0.0)
            wt2 = wp.tile([128, 1], G_DT, tag="wt2")
            nc.scalar.copy(wt2[:], wt[:])
            for b in range(BPC):
                x1a = xp.tile([KA, C], F32R, tag="x1a")
                x1b = xp.tile([KB, C], F32R, tag="x1b")
                x2a = xp.tile([KA, C], F32R, tag="x2a")
                x2b = xp.tile([KB, C], F32R, tag="x2b")
                nc.sync.dma_start(x1a[:], x1[b, 0:KA, :])
                nc.sync.dma_start(x2a[:], x2[b, 0:KA, :])
                nc.sync.dma_start(x1b[:], x1[b, KA:HW, :])
                nc.sync.dma_start(x2b[:], x2[b, KA:HW, :])
                pts = []
                for m in range(C // 128):
                    pt = ps.tile([128, C], F32)
                    nc.tensor.matmul(pt[:], x1a[:, m * 128:(m + 1) * 128],
                                     x2a[:], start=True, stop=False)
                    nc.tensor.matmul(pt[:], x1b[:, m * 128:(m + 1) * 128],
                                     x2b[:], start=False, stop=True)
                    pts.append(pt)
                # down-converts split DVE/Act into one per-sample tile;
                # a single store per sample on the Act queue.
                gt = gp.tile([128, C // 128, C], G_DT, tag="gt")
                for m in range(C // 128):
                    if m % 2 == 0:
                        nc.vector.tensor_copy(gt[:, m, :], pts[m][:])
                    else:
                        nc.scalar.copy(gt[:, m, :], pts[m][:])
                    if m % 2 == 1:  # store each finished half right away
                        nc.scalar.dma_start(g[b, :, m - 1:m + 1, :],
                                            gt[:, m - 1:m + 1, :])
    nc.compile()
    return nc


def _build_phase2(caps):
    """Per core: for each capacity class c (tiles share cap), input
    t{i} [128, nt_i, B, cap_i] bf16 (bucket-tile-major padded pair
    values, signs pre-folded into the bf16 sign bit) ->
    out [128, NTILES, B] f32 = per-bucket slot sums."""
    caps = list(caps)
    # group equal-cap consecutive tiles into classes
    classes = []  # (cap, ntiles)
    for cp in caps:
        if classes and classes[-1][0] == cp:
            classes[-1][1] += 1
        else:
            classes.append([cp, 1])

    nc = bacc.Bacc("TRN2", target_bir_lowering=False, debug=False,
                   num_devices=NCORES)
    tin = []
    for i, (cp, nt) in enumerate(classes):
        tin.append(nc.dram_tensor(f"t{i}", [128, nt, B, cp], G_DT,
                                  kind="ExternalInput").ap())
    out = nc.dram_tensor("out", [128, NTILES, B], F32,
                         kind="ExternalOutput").ap()

    # pipeline chunks of <=2 tiles: (class idx, tile offset within class,
    # ntiles, cap, global tile offset)
    chunks = []
    j0 = 0
    for i, (cp, nt) in enumerate(classes):
        for t0 in range(0, nt, 2):
            w = min(2, nt - t0)
            chunks.append((i, t0, w, cp, j0 + t0))
        j0 += nt
    # small DMA first (fills pipe fast), small fold-work last (short tail):
    # split the cheapest remaining chunk into single tiles for the tail
    first = min(chunks, key=lambda ch: ch[2] * ch[3])
    rest = [ch for ch in chunks if ch is not first]
    rest.sort(key=lambda ch: -ch[2] * ch[3])
    if rest and rest[-1][2] == 2:
        i, t0, w, cp, jg = rest.pop()
        rest += [(i, t0, 1, cp, jg), (i, t0 + 1, 1, cp, jg + 1)]
    chunks = [first] + rest

    with TileContext(nc) as tc:
        with (
            tc.tile_pool(name="tb", bufs=1) as tb,
            tc.tile_pool(name="ob", bufs=1) as ob,
        ):
            for ci, (i, t0, w, cp, jg) in enumerate(chunks):
                tt = tb.tile([128, w, B, cp], G_DT, tag=f"tt{ci}")
                nc.sync.dma_start(tt[:], tin[i][:, t0:t0 + w])
                c = cp
                while c % 2 == 0 and c > 4:
                    h = c // 2
                    nc.vector.tensor_tensor(
                        tt[:, :, :, 0:h], tt[:, :, :, 0:h], tt[:, :, :, h:c],
                        op=mybir.AluOpType.add)
                    c = h
                ro = ob.tile([128, w, B], F32, tag=f"ro{ci}")
                nc.vector.tensor_reduce(
                    ro[:], tt[:, :, :, 0:c],
                    axis=mybir.AxisListType.X, op=mybir.AluOpType.add)
                # per-chunk store on the otherwise-idle Act queue
                nc.scalar.dma_start(out[:, jg:jg + w, :], ro[:])
    nc.compile()
    return nc


def _run(nc, in_maps):
    _last_runs.append((nc, in_maps))
    res = bass_utils.run_bass_kernel_spmd(nc, in_maps,
                                          core_ids=list(range(NCORES)))
    return res.results


def _plan_tables(rand_h1, rand_s1, rand_h2, rand_s2):
    """Pure index bookkeeping (no float math on data).

    Buckets are sorted by occupancy and dealt round-robin: tile j of
    core k covers global occupancy-ranks j*1024 + 8*p + k, p in [0,128).
    Per-sample table layout per core: concat over capacity classes of
    [128, nt, cap] (sample dim inserted at kernel() time).

    Returns (caps, cls_shapes, src, neg, order):
      caps:       per-tile slot capacities (same for all cores).
      cls_shapes: [(ntiles, cap)] per capacity class.
      src:        int64 [NCORES, slots_ps] gather map: flat pair index
                  (c1*512+c2) + 1 per table slot, 0 for padding (index
                  into a zero-prefixed value array).
      neg:        bool [NCORES, slots_ps] negative-sign mask per slot.
      order:      int64 [NCORES, DPC] bucket id at (tile*128 + p).
    """
    h1 = rand_h1.astype(np.int64)
    h2 = rand_h2.astype(np.int64)
    bucket = ((h1[:, None] + h2[None, :]) % D).ravel()
    pos = (rand_s1[:, None] == rand_s2[None, :]).ravel()

    n = np.bincount(bucket, minlength=D)
    border = np.argsort(-n, kind="stable")       # occupancy rank -> bucket
    rank_of = np.empty(D, np.int64)
    rank_of[border] = np.arange(D)

    # per-tile caps from the global sorted chunks (identical across cores)
    ns = n[border]
    caps = [max(8, -(-int(ns[j * 1024:(j + 1) * 1024].max()) // 8) * 8)
            for j in range(NTILES)]

    # capacity classes: runs of equal cap (sorted desc -> contiguous)
    cls = []
    i0 = 0
    while i0 < NTILES:
        i1 = i0
        while i1 < NTILES and caps[i1] == caps[i0]:
            i1 += 1
        cls.append((i0, i1, caps[i0]))
        i0 = i1
    cls_base = np.zeros(NTILES, np.int64)   # per-sample class base offset
    cls_nt = np.zeros(NTILES, np.int64)
    cls_rel = np.zeros(NTILES, np.int64)
    cbase = 0
    for (i0, i1, cp) in cls:
        nt = i1 - i0
        for jj in range(i0, i1):
            cls_base[jj] = cbase
            cls_nt[jj] = nt
            cls_rel[jj] = jj - i0
        cbase += 128 * nt * cp
    slots_ps = cbase                        # per-sample slots per core

    # per-pair destination
    r = rank_of[bucket]                     # global occupancy rank
    j = r // 1024                           # tile index
    q = r % 1024
    core = q % 8
    p = q // 8                              # partition within tile
    # slot within bucket (stable order over pairs)
    o = np.argsort(r, kind="stable")
    s = np.empty(len(r), np.int64)
    rs = r[o]
    s[o] = np.arange(len(r)) - np.searchsorted(rs, rs)
    capv = np.asarray(caps, np.int64)[j]
    # class-major [128, nt, cap] flat position (per sample)
    flat = cls_base[j] + (p * cls_nt[j] + cls_rel[j]) * capv + s

    src = np.zeros((NCORES, slots_ps), np.int64)
    neg = np.zeros((NCORES, slots_ps), bool)
    pair_idx = np.arange(len(r), dtype=np.int64)
    for k in range(NCORES):
        m = core == k
        src[k, flat[m]] = pair_idx[m] + 1
        neg[k, flat[m]] = ~pos[m]

    order = np.empty((NCORES, DPC), np.int64)
    for k in range(NCORES):
        rr = (np.arange(NTILES)[:, None] * 1024
              + np.arange(128)[None, :] * 8 + k)   # [nt, 128] global ranks
        order[k] = border[rr].reshape(-1)          # bucket at (j*128+p)

    cls_shapes = [(i1 - i0, cp) for (i0, i1, cp) in cls]
    return caps, cls_shapes, src, neg, order


def kernel(bottom1, bottom2, rand_h1, rand_s1, rand_h2, rand_s2):
    _last_runs.clear()
    out_dtype = bottom1.dtype

    # ---- host: layout only (transpose / shard) ----
    x1 = np.ascontiguousarray(
        bottom1.transpose(0, 2, 3, 1).reshape(B, HW, C).astype(np.float32))
    x2 = np.ascontiguousarray(
        bottom2.transpose(0, 2, 3, 1).reshape(B, HW, C).astype(np.float32))

    pkey = (rand_h1.tobytes(), rand_s1.tobytes(),
            rand_h2.tobytes(), rand_s2.tobytes())
    if pkey not in _plan_cache:
        _plan_cache.clear()
        _plan_cache[pkey] = _plan_tables(
            np.asarray(rand_h1), np.asarray(rand_s1),
            np.asarray(rand_h2), np.asarray(rand_s2))
    caps, cls_shapes, src, neg, order = _plan_cache[pkey]

    # ---- phase 1: gram matrices ----
    if "p1" not in _cache:
        _cache["p1"] = _build_phase1()
    in_maps1 = [{"x1": x1[k * BPC:(k + 1) * BPC],
                 "x2": x2[k * BPC:(k + 1) * BPC]} for k in range(NCORES)]
    res1 = _run(_cache["p1"], in_maps1)
    # un-interleave [BPC, 128, 4, C] -> [BPC, 512, C] (layout only)
    g_all = np.concatenate(
        [np.ascontiguousarray(r["g"].transpose(0, 2, 1, 3)).reshape(BPC, C, C)
         for r in res1], axis=0)                            # [B, C, C] bf16

    # ---- host: reshard pairs into bucket-major capacity-class tables ----
    g_pairs = g_all.reshape(B, C * C)                     # [B, pairs] bf16
    # zero-prefixed so src==0 (padding) gathers 0.0
    gz = np.concatenate(
        [np.zeros((B, 1), g_pairs.dtype), g_pairs], axis=1)

    key = ("p2", tuple(caps))
    if key not in _cache:
        _cache[key] = _build_phase2(caps)

    in_maps2 = []
    for k in range(NCORES):
        vals = gz[:, src[k]]                              # [B, slots_ps]
        # sign-bit flip is part of the count-sketch hash (no FLOPs)
        vals.view(np.uint16)[:, neg[k]] ^= np.uint16(0x8000)
        m = {}
        off = 0
        for i, (nt, cp) in enumerate(cls_shapes):
            sz = 128 * nt * cp
            # [B, 128, nt, cap] -> [128, nt, B, cap]
            blk = vals[:, off:off + sz].reshape(B, 128, nt, cp)
            m[f"t{i}"] = np.ascontiguousarray(blk.transpose(1, 2, 0, 3))
            off += sz
        in_maps2.append(m)
    res2 = _run(_cache[key], in_maps2)

    # per-core out [128, NTILES, B] -> scatter back to bucket ids
    out = np.empty((B, D), np.float32)
    for k in range(NCORES):
        ok = res2[k]["out"].transpose(1, 0, 2).reshape(DPC, B)  # (j*128+p, b)
        out[:, order[k]] = ok.T
    return np.ascontiguousarray(out).astype(out_dtype)


# revision 16
# speedup vs baseline: 1.3734x; 1.0392x over previous
"""Compact Bilinear Pooling on 8 Trainium2 NeuronCores.

Math: for each sample b, Output[b] = sum over pixels p of
  countsketch(x1_p) (circular-conv) countsketch(x2_p)
which, because the sum over pixels commutes with the bilinear pair
products, equals a scatter-reduce of the per-sample gram matrix
  G_b[c1, c2] = sum_p x1[b,p,c1] * x2[b,p,c2]
into buckets d = (h1[c1] + h2[c2]) mod 8192 with signs s1[c1]*s2[c2].

Device plan (two launches, both index-independent programs):
  Phase 1 (batch-sharded, 4 samples/core): G_b = X1_b^T @ X2_b on the
    tensor engine -> DRAM (bf16). PSUM->SBUF down-converts alternate
    between the vector (DVE) and scalar (Activation) engines so they
    hide under the DMA stream; each output DMA is issued from the
    engine that produced the tile, so the SP queue never stalls.
  Host: zero-FLOP reshard. Pair->bucket map is compile-time data; the
    8192 buckets are sorted by occupancy and dealt round-robin to the
    8 cores (identical SPMD tables + perfect balance), then grouped
    into 128-bucket tiles whose capacity is the tile max rounded up to
    a multiple of 8 (keeps every DMA descriptor >= 512B). Sketch signs
    are folded in as a sign-bit flip (part of the hash, not FLOPs).
  Phase 2 (bucket-sharded, 1024 buckets/core): per capacity class, a
    bf16 fold tree (tensor_tensor add, 2x DVE mode) halves the slot
    dim while it is even and > 4, then one tensor_reduce (f32 accum)
    finishes the remaining 3..7 slots.
"""

import numpy as np

import concourse.bass as bass
import concourse.bacc as bacc
import concourse.mybir as mybir
from concourse.tile import TileContext
from concourse import bass_utils

B, C, HW, D = 32, 512, 196, 8192
NCORES = 8
BPC = B // NCORES          # samples per core in phase 1
DPC = D // NCORES          # buckets per core in phase 2
NTILES = DPC // 128        # 128-bucket tiles per core in phase 2
F32 = mybir.dt.float32
F32R = mybir.dt.float32r   # TF32-like PE mode: 1 cycle/row vs 4 for fp32
BF16 = mybir.dt.bfloat16
G_DT = BF16                # gram matrix precision on the wire

_cache = {}
_plan_cache = {}
_last_runs = []  # (nc, in_maps) of the most recent kernel() call, for profiling


def _build_phase1():
    """Per core: x1,x2 [BPC, 196, 512] f32 -> g [BPC, 512, 512] bf16."""
    nc = bacc.Bacc("TRN2", target_bir_lowering=False, debug=False,
                   num_devices=NCORES)
    x1 = nc.dram_tensor("x1", [BPC, HW, C], F32R, kind="ExternalInput").ap()
    x2 = nc.dram_tensor("x2", [BPC, HW, C], F32R, kind="ExternalInput").ap()
    # partition-major interleave: g[b, p, m, c] = G[b, m*128+p, c]
    # (host un-interleaves; pure layout)
    g = nc.dram_tensor("g", [BPC, 128, C // 128, C], G_DT,
                       kind="ExternalOutput").ap()

    KA, KB = 128, HW - 128  # pixel (contraction) dim split

    with TileContext(nc) as tc:
        with (
            tc.tile_pool(name="xp", bufs=3) as xp,
            tc.tile_pool(name="gp", bufs=3) as gp,
            tc.tile_pool(name="ps", bufs=8, space="PSUM") as ps,
            tc.tile_pool(name="wp", bufs=1) as wp,
        ):
            # pre-warm the Act function table so the 1.3us LoadActFuncSet
            # hides under the first input loads
            wt = wp.tile([128, 1], F32, tag="wt")
            nc.gpsimd.memset(wt[:], 0.0)
            wt2 = wp.tile([128, 1], G_DT, tag="wt2")
            nc.scalar.copy(wt2[:], wt[:])
            for b in range(BPC):
                x1a = xp.tile([KA, C], F32R, tag="x1a")
                x1b = xp.tile([KB, C], F32R, tag="x1b")
                x2a = xp.tile([KA, C], F32R, tag="x2a")
                x2b = xp.tile([KB, C], F32R, tag="x2b")
                nc.sync.dma_start(x1a[:], x1[b, 0:KA, :])
                nc.sync.dma_start(x2a[:], x2[b, 0:KA, :])
                nc.sync.dma_start(x1b[:], x1[b, KA:HW, :])
                nc.sync.dma_start(x2b[:], x2[b, KA:HW, :])
                pts = []
                for m in range(C // 128):
                    pt = ps.tile([128, C], F32)
                    nc.tensor.matmul(pt[:], x1a[:, m * 128:(m + 1) * 128],
                                     x2a[:], start=True, stop=False)
                    nc.tensor.matmul(pt[:], x1b[:, m * 128:(m + 1) * 128],
                                     x2b[:], start=False, stop=True)
                    pts.append(pt)
                # down-converts split DVE/Act into one per-sample tile;
                # a single store per sample on the Act queue.
                gt = gp.tile([128, C // 128, C], G_DT, tag="gt")
                for m in range(C // 128):
                    # Act is the busier queue (it also issues the stores):
                    # give it only one copy in four.
                    if m == 3:
                        nc.scalar.copy(gt[:, m, :], pts[m][:])
                    else:
                        nc.vector.tensor_copy(gt[:, m, :], pts[m][:])
                    if m % 2 == 1:  # store each finished half right away
                        nc.scalar.dma_start(g[b, :, m - 1:m + 1, :],
                                            gt[:, m - 1:m + 1, :])
    nc.compile()
    return nc


def _build_phase2(caps):
    """Per core: for each capacity class c (tiles share cap), input
    t{i} [128, nt_i, B, cap_i] bf16 (bucket-tile-major padded pair
    values, signs pre-folded into the bf16 sign bit) ->
    out [128, NTILES, B] f32 = per-bucket slot sums."""
    caps = list(caps)
    # group equal-cap consecutive tiles into classes
    classes = []  # (cap, ntiles)
    for cp in caps:
        if classes and classes[-1][0] == cp:
            classes[-1][1] += 1
        else:
            classes.append([cp, 1])

    nc = bacc.Bacc("TRN2", target_bir_lowering=False, debug=False,
                   num_devices=NCORES)
    tin = []
    for i, (cp, nt) in enumerate(classes):
        tin.append(nc.dram_tensor(f"t{i}", [128, nt, B, cp], G_DT,
                                  kind="ExternalInput").ap())
    out = nc.dram_tensor("out", [128, NTILES, B], F32,
                         kind="ExternalOutput").ap()

    # pipeline chunks of <=2 tiles: (class idx, tile offset within class,
    # ntiles, cap, global tile offset)
    chunks = []
    j0 = 0
    for i, (cp, nt) in enumerate(classes):
        for t0 in range(0, nt, 2):
            w = min(2, nt - t0)
            chunks.append((i, t0, w, cp, j0 + t0))
        j0 += nt
    # small DMA first (fills pipe fast), small fold-work last (short tail):
    # split the cheapest remaining chunk into single tiles for the tail
    first = min(chunks, key=lambda ch: ch[2] * ch[3])
    rest = [ch for ch in chunks if ch is not first]
    rest.sort(key=lambda ch: -ch[2] * ch[3])
    chunks = [first] + rest

    with TileContext(nc) as tc:
        with (
            tc.tile_pool(name="tb", bufs=1) as tb,
            tc.tile_pool(name="ob", bufs=1) as ob,
        ):
            for ci, (i, t0, w, cp, jg) in enumerate(chunks):
                tt = tb.tile([128, w, B, cp], G_DT, tag=f"tt{ci}")
                nc.sync.dma_start(tt[:], tin[i][:, t0:t0 + w])
                c = cp
                while c % 2 == 0 and c > 4:
                    h = c // 2
                    nc.vector.tensor_tensor(
                        tt[:, :, :, 0:h], tt[:, :, :, 0:h], tt[:, :, :, h:c],
                        op=mybir.AluOpType.add)
                    c = h
                ro = ob.tile([128, w, B], F32, tag=f"ro{ci}")
                nc.vector.tensor_reduce(
                    ro[:], tt[:, :, :, 0:c],
                    axis=mybir.AxisListType.X, op=mybir.AluOpType.add)
                # per-chunk store on the otherwise-idle Act queue
                nc.scalar.dma_start(out[:, jg:jg + w, :], ro[:])
    nc.compile()
    return nc


def _run(nc, in_maps):
    _last_runs.append((nc, in_maps))
    res = bass_utils.run_bass_kernel_spmd(nc, in_maps,
                                          core_ids=list(range(NCORES)))
    return res.results


def _plan_tables(rand_h1, rand_s1, rand_h2, rand_s2):
    """Pure index bookkeeping (no float math on data).

    Buckets are sorted by occupancy and dealt round-robin: tile j of
    core k covers global occupancy-ranks j*1024 + 8*p + k, p in [0,128).
    Per-sample table layout per core: concat over capacity classes of
    [128, nt, cap] (sample dim inserted at kernel() time).

    Returns (caps, cls_shapes, src, neg, order):
      caps:       per-tile slot capacities (same for all cores).
      cls_shapes: [(ntiles, cap)] per capacity class.
      src:        int64 [NCORES, slots_ps] gather map: flat pair index
                  (c1*512+c2) + 1 per table slot, 0 for padding (index
                  into a zero-prefixed value array).
      neg:        bool [NCORES, slots_ps] negative-sign mask per slot.
      order:      int64 [NCORES, DPC] bucket id at (tile*128 + p).
    """
    h1 = rand_h1.astype(np.int64)
    h2 = rand_h2.astype(np.int64)
    bucket = ((h1[:, None] + h2[None, :]) % D).ravel()
    pos = (rand_s1[:, None] == rand_s2[None, :]).ravel()

    n = np.bincount(bucket, minlength=D)
    border = np.argsort(-n, kind="stable")       # occupancy rank -> bucket
    rank_of = np.empty(D, np.int64)
    rank_of[border] = np.arange(D)

    # per-tile caps from the global sorted chunks (identical across cores)
    ns = n[border]
    caps = [max(8, -(-int(ns[j * 1024:(j + 1) * 1024].max()) // 8) * 8)
            for j in range(NTILES)]

    # capacity classes: runs of equal cap (sorted desc -> contiguous)
    cls = []
    i0 = 0
    while i0 < NTILES:
        i1 = i0
        while i1 < NTILES and caps[i1] == caps[i0]:
            i1 += 1
        cls.append((i0, i1, caps[i0]))
        i0 = i1
    cls_base = np.zeros(NTILES, np.int64)   # per-sample class base offset
    cls_nt = np.zeros(NTILES, np.int64)
    cls_rel = np.zeros(NTILES, np.int64)
    cbase = 0
    for (i0, i1, cp) in cls:
        nt = i1 - i0
        for jj in range(i0, i1):
            cls_base[jj] = cbase
            cls_nt[jj] = nt
            cls_rel[jj] = jj - i0
        cbase += 128 * nt * cp
    slots_ps = cbase                        # per-sample slots per core

    # per-pair destination
    r = rank_of[bucket]                     # global occupancy rank
    j = r // 1024                           # tile index
    q = r % 1024
    core = q % 8
    p = q // 8                              # partition within tile
    # slot within bucket (stable order over pairs)
    o = np.argsort(r, kind="stable")
    s = np.empty(len(r), np.int64)
    rs = r[o]
    s[o] = np.arange(len(r)) - np.searchsorted(rs, rs)
    capv = np.asarray(caps, np.int64)[j]
    # class-major [128, nt, cap] flat position (per sample)
    flat = cls_base[j] + (p * cls_nt[j] + cls_rel[j]) * capv + s

    src = np.zeros((NCORES, slots_ps), np.int64)
    neg = np.zeros((NCORES, slots_ps), bool)
    pair_idx = np.arange(len(r), dtype=np.int64)
    for k in range(NCORES):
        m = core == k
        src[k, flat[m]] = pair_idx[m] + 1
        neg[k, flat[m]] = ~pos[m]

    order = np.empty((NCORES, DPC), np.int64)
    for k in range(NCORES):
        rr = (np.arange(NTILES)[:, None] * 1024
              + np.arange(128)[None, :] * 8 + k)   # [nt, 128] global ranks
        order[k] = border[rr].reshape(-1)          # bucket at (j*128+p)

    cls_shapes = [(i1 - i0, cp) for (i0, i1, cp) in cls]
    return caps, cls_shapes, src, neg, order


def kernel(bottom1, bottom2, rand_h1, rand_s1, rand_h2, rand_s2):
    _last_runs.clear()
    out_dtype = bottom1.dtype

    # ---- host: layout only (transpose / shard) ----
    x1 = np.ascontiguousarray(
        bottom1.transpose(0, 2, 3, 1).reshape(B, HW, C).astype(np.float32))
    x2 = np.ascontiguousarray(
        bottom2.transpose(0, 2, 3, 1).reshape(B, HW, C).astype(np.float32))

    pkey = (rand_h1.tobytes(), rand_s1.tobytes(),
            rand_h2.tobytes(), rand_s2.tobytes())
    if pkey not in _plan_cache:
        _plan_cache.clear()
        _plan_cache[pkey] = _plan_tables(
            np.asarray(rand_h1), np.asarray(rand_s1),
            np.asarray(rand_h2), np.asarray(rand_s2))
    caps, cls_shapes, src, neg, order = _plan_cache[pkey]

    # ---- phase 1: gram matrices ----
    if "p1" not in _cache:
        _cache["p1"] = _build_phase1()
    in_maps1 = [{"x1": x1[k * BPC:(k + 1) * BPC],
                 "x2": x2[k * BPC:(k + 1) * BPC]} for k in range(NCORES)]
    res1 = _run(_cache["p1"], in_maps1)
    # un-interleave [BPC, 128, 4, C] -> [BPC, 512, C] (layout only)
    g_all = np.concatenate(
        [np.ascontiguousarray(r["g"].transpose(0, 2, 1, 3)).reshape(BPC, C, C)
         for r in res1], axis=0)                            # [B, C, C] bf16

    # ---- host: reshard pairs into bucket-major capacity-class tables ----
    g_pairs = g_all.reshape(B, C * C)                     # [B, pairs] bf16
    # zero-prefixed so src==0 (padding) gathers 0.0
    gz = np.concatenate(
        [np.zeros((B, 1), g_pairs.dtype), g_pairs], axis=1)

    key = ("p2", tuple(caps))
    if key not in _cache:
        _cache[key] = _build_phase2(caps)

    in_maps2 = []
    for k in range(NCORES):
        vals = gz[:, src[k]]                              # [B, slots_ps]
        # sign-bit flip is part of the count-sketch hash (no FLOPs)
        vals.view(np.uint16)[:, neg[k]] ^= np.uint16(0x8000)
        m = {}
        off = 0
        for i, (nt, cp) in enumerate(cls_shapes):
            sz = 128 * nt * cp
            # [B, 128, nt, cap] -> [128, nt, B, cap]
            blk = vals[:, off:off + sz].reshape(B, 128, nt, cp)
            m[f"t{i}"] = np.ascontiguousarray(blk.transpose(1, 2, 0, 3))
            off += sz
        in_maps2.append(m)
    res2 = _run(_cache[key], in_maps2)

    # per-core out [128, NTILES, B] -> scatter back to bucket ids
    out = np.empty((B, D), np.float32)
    for k in range(NCORES):
        ok = res2[k]["out"].transpose(1, 0, 2).reshape(DPC, B)  # (j*128+p, b)
        out[:, order[k]] = ok.T
    return np.ascontiguousarray(out).astype(out_dtype)
